# revision 6
# baseline (speedup 1.0000x reference)
"""Block2D attention on 8 TRN2 NeuronCores (fp16 compute, fp32 accum).

Sharding: data-parallel over the 8 independent (b, bnx, bny) attention blocks
(B=2 x bnx=2 x bny=2), one block of T=1024 tokens per core. Blocks are fully
independent so no collectives are needed; each core runs the whole
qkv-projection -> block attention -> output projection chain for its block.

Per-core pipeline (fp16 matmuls, fp32 PSUM accumulation):
  setup: xT/wv/wk/wq streamed per k-tile up front; V projection runs k-major
         in two 4-bank PSUM passes so compute starts after the first k-tile
         lands and the K projection's PSUM frees early; K projection with
         pair-0's q projection interleaved; kT duplicated into kdup per group.
  pair loop (heads 2j, 2j+1 per pair j, kv group g=j//2), software-pipelined
  in 16 steps per pair (u = hh*8 + t):
    step u: QK(u): scoresT[t-tile, hh] = k_g @ q_h^T  (2 matmuls, psum ping-pong)
            exp(u): ACT e = exp(scale*s) -> fp16      (paced 1:1 with QK)
            PV(u-2): two accumulating matmuls into pv psum (unit lags 2 steps
                     so the exp result is ready; denominator in psum row 64)
            Q(j+1): 2 of next pair's 32 q-projection matmuls
    This keeps PE and ACT both continuously busy instead of the bursty
    QK-then-PV phases that stall each on the other.
  normalization: rec = 1/den (DVE), DRAM-bounce broadcast -> rbc;
    oT[:, j, :] = o_unnorm * rbc on GPSIMD, off the critical path.
  out = oT^T @ Wo -> [1024, 2048] fp32 rows [blk*1024:(blk+1)*1024].
"""

import os
import sys

sys.path.insert(0, "/opt/trn_rl_repo")

import numpy as np
import ml_dtypes

import concourse.bass as bass
from concourse import bacc
import concourse.mybir as mybir
import concourse.tile as tile

F32 = mybir.dt.float32
BF16 = mybir.dt.float16   # compute dtype: fp16 (same PE speed as bf16, 8x finer mantissa)
BF = np.float16

H = 2048        # hidden
T = 1024        # tokens per block
NH = 32         # q heads
NKV = 8         # kv heads
D = 64          # head dim
KT = H // 128   # 16 hidden k-tiles
TT = T // 128   # 8 token tiles
NPAIR = NH // 2  # 16 head pairs
SCALE = D ** -0.5

LAST_EXEC_TIME_NS = None
LAST_RESULTS = None
_CACHED_NC = None


def build_nc(e_bufs=5, wq_bufs=3, qp_bufs=2, qk_bufs=2, pv_bufs=2, ou_bufs=4):
    nc = bacc.Bacc("TRN2")
    xT = nc.dram_tensor("xT", [H, T], BF16, kind="ExternalInput")
    wq = nc.dram_tensor("wq", [H, H], BF16, kind="ExternalInput")
    wk = nc.dram_tensor("wk", [H, NKV * D], BF16, kind="ExternalInput")
    wv = nc.dram_tensor("wv", [H, NKV * D], BF16, kind="ExternalInput")
    wo = nc.dram_tensor("wo", [H, H], BF16, kind="ExternalInput")
    out = nc.dram_tensor("out", [T, H], F32, kind="ExternalOutput")
    scr = nc.dram_tensor("scr", [NPAIR, 2, 2, 512], F32)  # recip bounce [j, hh, n]

    xT_v = xT.ap().rearrange("(k p) t -> p k t", p=128)
    wq_v = wq.ap().rearrange("(k p) m -> p k m", p=128)
    wk_v = wk.ap().rearrange("(k p) m -> p k m", p=128)
    wv_v = wv.ap().rearrange("(k p) m -> p k m", p=128)
    wo_v = wo.ap().rearrange("(k p) m -> p k m", p=128)

    with tile.TileContext(nc) as tc:
        with (
            tc.tile_pool(name="oT", bufs=1) as oT_pool,
            tc.tile_pool(name="xTs", bufs=1) as xT_pool,
            tc.tile_pool(name="kdup", bufs=1) as kdup_pool,
            tc.tile_pool(name="vplus", bufs=1) as vplus_pool,
            tc.tile_pool(name="wos", bufs=2) as wo_pool,
            tc.tile_pool(name="wks", bufs=1) as wk_pool,  # two halves share one slot
            tc.tile_pool(name="wqs", bufs=wq_bufs) as wq_pool,
            tc.tile_pool(name="qpair", bufs=3) as qpair_pool,
            tc.tile_pool(name="e", bufs=e_bufs) as e_pool,
            tc.tile_pool(name="ou", bufs=ou_bufs) as ou_pool,
            tc.tile_pool(name="rec", bufs=2) as rec_pool,
            tc.tile_pool(name="rbc", bufs=2) as rbc_pool,
        ):
            oT = oT_pool.tile([128, KT, T], BF16)
            xTs = xT_pool.tile([128, KT, T], BF16)
            kdup = kdup_pool.tile([128, NKV, T], BF16)  # k_g^T on both halves
            vplus = vplus_pool.tile([128, TT, NKV, D + 1], BF16)
            wk_halves = [None, None]

            def dma_wk(h):
                wkh = wk_pool.tile([128, KT, 2, 128], BF16, tag="wk", name="wk")
                nc.sync.dma_start(
                    out=wkh,
                    in_=wk_v.rearrange("p k (hh mm m) -> p k hh mm m", hh=2, mm=2)[
                        :, :, h
                    ],
                )
                wk_halves[h] = wkh

            wq_tiles = [None] * (NPAIR + 1)
            qpair_tiles = [None] * (NPAIR + 1)

            def dma_wq(j):
                wq_s = wq_pool.tile([128, KT, 128], BF16, tag="wq_s", name="wq_s")
                nc.sync.dma_start(out=wq_s, in_=wq_v[:, :, 128 * j:128 * (j + 1)])
                wq_tiles[j] = wq_s
                qpair_tiles[j] = qpair_pool.tile(
                    [128, T], BF16, tag="qpair", name="qpair"
                )

            # ---- input DMA stream: per-k-tile, compute starts on tile 0 ----
            with tc.tile_pool(name="wvs", bufs=1) as wv_pool:
                wv_s = wv_pool.tile([128, KT, 512], BF16)
                for k in range(KT):
                    nc.sync.dma_start(out=wv_s[:, k:k + 1, :], in_=wv_v[:, k:k + 1, :])
                    nc.sync.dma_start(out=xTs[:, k:k + 1, :], in_=xT_v[:, k:k + 1, :])
                dma_wk(0)
                for j in range(3):
                    dma_wq(j)

                # ---- V projection, k-major, two 4-bank passes ----
                for half in range(2):
                    with tc.tile_pool(name=f"vps{half}", bufs=1, space="PSUM") as vps:
                        ms = range(4 * half, 4 * half + 4)
                        vtiles = {
                            m: vps.tile([128, 512], F32, tag=f"v{m}", name=f"v{m}")
                            for m in ms
                        }
                        for k in range(KT):
                            for m in ms:
                                nc.tensor.matmul(
                                    vtiles[m], xTs[:, k, 128 * m:128 * (m + 1)],
                                    wv_s[:, k, :],
                                    start=(k == 0), stop=(k == KT - 1),
                                )
                        for m in ms:
                            nc.vector.tensor_copy(
                                vplus[:, m, :, 0:D],
                                vtiles[m].rearrange("p (h d) -> p h d", h=NKV),
                            )
                nc.vector.memset(vplus[:, :, :, D:D + 1], 1.0)

            # ---- K projection with pair-0 q projection interleaved ----
            q_ps = [None] * (NPAIR + 1)       # open q-proj psum tile per pair

            def q_mm(j, i, pool, tag):
                """Emit the i-th (of 32) q-projection matmul for pair j."""
                n, k = divmod(i, KT)
                if k == 0:
                    q_ps[j] = pool.tile([128, 512], F32, tag=tag, name="qps")
                nc.tensor.matmul(
                    q_ps[j], wq_tiles[j][:, k, :],
                    xTs[:, k, 512 * n:512 * (n + 1)],
                    start=(k == 0), stop=(k == KT - 1),
                )
                if k == KT - 1:
                    nc.vector.tensor_copy(
                        qpair_tiles[j][:, 512 * n:512 * (n + 1)], q_ps[j]
                    )

            with (
                tc.tile_pool(name="kTs", bufs=1) as kT_pool,
                tc.tile_pool(name="kps", bufs=2, space="PSUM") as kps,
            ):
                kTs = kT_pool.tile([128, 4, T], BF16)
                q0 = 0
                for m in range(4):
                    if m == 2:
                        dma_wk(1)
                    for n in range(2):
                        ps = kps.tile([128, 512], F32, tag="kps", name="kps")
                        for k in range(KT):
                            nc.tensor.matmul(
                                ps, wk_halves[m // 2][:, k, m % 2, :],
                                xTs[:, k, 512 * n:512 * (n + 1)],
                                start=(k == 0), stop=(k == KT - 1),
                            )
                        nc.vector.tensor_copy(kTs[:, m, 512 * n:512 * (n + 1)], ps)
                        for _ in range(4):
                            q_mm(0, q0, kps, "q0ps")
                            q0 += 1
                    for gg in range(2):
                        g = 2 * m + gg
                        src = kTs[64 * gg:64 * gg + 64, m, :]
                        nc.sync.dma_start(out=kdup[0:64, g, :], in_=src)
                        nc.sync.dma_start(out=kdup[64:128, g, :], in_=src)

            # ---------------- pair loop (software-pipelined) ----------------
            with (
                tc.tile_pool(name="pp", bufs=qp_bufs, space="PSUM") as pp,
                tc.tile_pool(name="qk", bufs=1, space="PSUM") as qk_pool,
                tc.tile_pool(name="pv", bufs=pv_bufs, space="PSUM") as pv_pool,
            ):
                wo_tiles = [None] * 4
                wo_s0 = wo_pool.tile([128, KT, 512], BF16, tag="wo_s")
                nc.sync.dma_start(out=wo_s0, in_=wo_v[:, :, 0:512])
                wo_tiles[0] = wo_s0

                # persistent alternating qk psum tiles: unit u writes
                # qk_ring[u % qk_bufs]; the WAR edge is exactly vs the exp
                # that read it qk_bufs units ago.
                qk_ring = [
                    qk_pool.tile([128, T], F32, tag=f"qkr{i}", name=f"qkr{i}")
                    for i in range(qk_bufs)
                ]

                e_tiles = {}                       # (j, u) -> e tile
                pv_tiles = {}                      # (j, hh) -> [pv_n0, pv_n1]
                ou_tiles = {}                      # (j, n) -> packed ou tile

                def qk_exp(j, u):
                    hh, t = divmod(u, 8)
                    g = j // 2
                    lo, hi = 64 * hh, 64 * hh + 64
                    qk = qk_ring[(16 * j + u) % qk_bufs]
                    for n in range(2):
                        nc.tensor.matmul(
                            qk[:, 512 * n:512 * (n + 1)],
                            kdup[lo:hi, g, 128 * t:128 * (t + 1)],
                            qpair_tiles[j][lo:hi, 512 * n:512 * (n + 1)],
                            start=True, stop=True,
                        )
                    e = e_pool.tile([128, T], BF16, tag="e", name="e")
                    nc.scalar.activation(
                        e, qk, mybir.ActivationFunctionType.Exp, scale=SCALE
                    )
                    e_tiles[(j, u)] = e

                def pv_mm(j, u):
                    hh, t = divmod(u, 8)
                    g = j // 2
                    if t == 0:
                        pv_tiles[(j, hh)] = [
                            pv_pool.tile([65, 512], F32, tag="pv", name=f"pv{n}")
                            for n in range(2)
                        ]
                    e = e_tiles.pop((j, u))
                    for n in range(2):
                        nc.tensor.matmul(
                            pv_tiles[(j, hh)][n], vplus[:, t, g, :],
                            e[:, 512 * n:512 * (n + 1)],
                            start=(t == 0), stop=(t == TT - 1),
                        )
                    if t == TT - 1:
                        finish_head(j, hh)

                def finish_head(j, hh):
                    """Drain pv psum for (j, hh): unnormalized o + denominator,
                    reciprocal via 128-wide DVE, DRAM-bounce broadcast, then
                    GPSIMD normalize into oT (off critical path)."""
                    lo, hi = 64 * hh, 64 * hh + 64
                    den = rec_pool.tile([1, 2 * 512], F32, tag="den", name="den")
                    for n in range(2):
                        pv = pv_tiles[(j, hh)][n]
                        if (j, n) not in ou_tiles:
                            ou_tiles[(j, n)] = ou_pool.tile(
                                [128, 512], F32, tag="ou", name=f"ou{n}"
                            )
                        nc.vector.tensor_copy(ou_tiles[(j, n)][lo:hi, :], pv[0:64, :])
                        nc.vector.tensor_copy(den[:, 512 * n:512 * (n + 1)], pv[64:65, :])
                    den_sp = rec_pool.tile([128, 8], F32, tag="den_sp", name="den_sp")
                    sp_src = bass.AP(
                        tensor=den.tensor, offset=den.offset,
                        ap=[[1, 1], [8, 128], [1, 8]],
                    )
                    nc.sync.dma_start(out=den_sp, in_=sp_src)
                    rec_sp = rec_pool.tile([128, 8], F32, tag="rec_sp", name="rec_sp")
                    nc.vector.reciprocal(rec_sp, den_sp)
                    nc.sync.dma_start(
                        out=scr.ap()[j, hh].rearrange("a b -> (a b)")
                        .rearrange("(p e) -> p e", p=128),
                        in_=rec_sp,
                    )
                    rbc = rbc_pool.tile([128, T], F32, tag="rbc", name="rbc")
                    bsrc = bass.AP(
                        tensor=scr.ap().tensor,
                        offset=(2 * j + hh) * T,
                        ap=[[0, 64], [1, 1024]],
                    )
                    nc.sync.dma_start(out=rbc[lo:hi, :], in_=bsrc)
                    for n in range(2):
                        nc.gpsimd.tensor_mul(
                            oT[lo:hi, j, 512 * n:512 * (n + 1)],
                            ou_tiles[(j, n)][lo:hi, :],
                            rbc[lo:hi, 512 * n:512 * (n + 1)],
                        )

                # steady state: 16 steps per pair; step u of pair j carries
                # QK(j,u) + exp, PV for the unit 2 steps back, and 2 matmuls
                # of pair j+1's q projection.
                for j in range(NPAIR):
                    if 3 <= j + 2 <= NPAIR - 1:
                        dma_wq(j + 2)
                    for u in range(16):
                        qk_exp(j, u)
                        s = 16 * j + u
                        if s >= 2:
                            jp, up = divmod(s - 2, 16)
                            pv_mm(jp, up)
                        if j + 1 <= NPAIR - 1:
                            q_mm(j + 1, 2 * u, pp, "pp")
                            q_mm(j + 1, 2 * u + 1, pp, "pp")
                # drain: last two pv units
                for s in range(16 * NPAIR, 16 * NPAIR + 2):
                    jp, up = divmod(s - 2, 16)
                    pv_mm(jp, up)

            # ---------------- output projection ----------------
            with (
                tc.tile_pool(name="ob", bufs=4) as ob_pool,
                tc.tile_pool(name="ops", bufs=4, space="PSUM") as ops,
            ):
                for c in range(4):
                    if wo_tiles[c] is None:
                        wo_s = wo_pool.tile([128, KT, 512], BF16, tag="wo_s")
                        nc.sync.dma_start(
                            out=wo_s, in_=wo_v[:, :, 512 * c:512 * (c + 1)]
                        )
                        wo_tiles[c] = wo_s
                    wo_s = wo_tiles[c]
                    for m in range(TT):
                        ps = ops.tile([128, 512], F32, tag="ops", name="ops")
                        for k in range(KT):
                            nc.tensor.matmul(
                                ps, oT[:, k, 128 * m:128 * (m + 1)], wo_s[:, k, :],
                                start=(k == 0), stop=(k == KT - 1),
                            )
                        ob = ob_pool.tile([128, 512], F32, tag="ob", name="ob")
                        nc.scalar.copy(ob, ps)
                        nc.sync.dma_start(
                            out=out.ap()[128 * m:128 * (m + 1), 512 * c:512 * (c + 1)],
                            in_=ob,
                        )
    nc.finalize()
    return nc


def _prep_inputs(hidden_states, Wq, Wk, Wv, Wo):
    hs = np.asarray(hidden_states, dtype=np.float32)
    B = hs.shape[0]
    # token index l = ix*2048 + sx*64 + iy*32 + sy  (bnx=2, BSX=32, bny=2, BSY=32)
    hsv = hs.reshape(B, 2, 32, 2, 32, H)  # b ix sx iy sy h
    wq_b = np.asarray(Wq, dtype=np.float32).astype(BF)
    wk_b = np.asarray(Wk, dtype=np.float32).astype(BF)
    wv_b = np.asarray(Wv, dtype=np.float32).astype(BF)
    wo_b = np.asarray(Wo, dtype=np.float32).astype(BF)
    in_maps = []
    for c in range(8):
        b, ix, iy = c // 4, (c // 2) % 2, c % 2
        x_blk = hsv[b, ix, :, iy, :, :].reshape(T, H)
        xT = np.ascontiguousarray(x_blk.T).astype(BF)
        in_maps.append({"xT": xT, "wq": wq_b, "wk": wk_b, "wv": wv_b, "wo": wo_b})
    return in_maps


def kernel(hidden_states, Wq, Wk, Wv, Wo, x_dim=64, y_dim=64):
    global LAST_EXEC_TIME_NS, LAST_RESULTS, _CACHED_NC
    assert int(x_dim) == 64 and int(y_dim) == 64

    from concourse.bass_utils import run_bass_kernel_spmd

    if _CACHED_NC is None:
        _CACHED_NC = build_nc()
    nc = _CACHED_NC

    in_maps = _prep_inputs(hidden_states, Wq, Wk, Wv, Wo)
    trace = bool(os.environ.get("BASS_TRACE"))
    res = run_bass_kernel_spmd(nc, in_maps, core_ids=list(range(8)), trace=trace)
    LAST_EXEC_TIME_NS = res.exec_time_ns
    LAST_RESULTS = res
    out = np.concatenate([r["out"] for r in res.results], axis=0)
    return np.ascontiguousarray(out.reshape(2, 4096, H).astype(np.float32))


# revision 7
# speedup vs baseline: 1.1904x; 1.1904x over previous
"""Block2D attention on 8 TRN2 NeuronCores (fp16 compute, fp32 accum).

Sharding: data-parallel over the 8 independent (b, bnx, bny) attention blocks
(B=2 x bnx=2 x bny=2), one block of T=1024 tokens per core. Blocks are fully
independent so no collectives are needed; each core runs the whole
qkv-projection -> block attention -> output projection chain for its block.

Per-core pipeline (fp16 matmuls, fp32 PSUM accumulation):
  setup: xT/wv/wk/wq streamed per k-tile up front; V projection runs k-major
         in two 4-bank PSUM passes so compute starts after the first k-tile
         lands and the K projection's PSUM frees early; K projection with
         pair-0's q projection interleaved; kT duplicated into kdup per group.
  pair loop (heads 2j, 2j+1 per pair j, kv group g=j//2), software-pipelined
  in 16 steps per pair (u = hh*8 + t):
    step u: QK(u): scoresT[t-tile, hh] = k_g @ q_h^T  (2 matmuls, psum ping-pong)
            exp(u): ACT e = exp(scale*s) -> fp16      (paced 1:1 with QK)
            PV(u-2): two accumulating matmuls into pv psum (unit lags 2 steps
                     so the exp result is ready; denominator in psum row 64)
            Q(j+1): 2 of next pair's 32 q-projection matmuls
    This keeps PE and ACT both continuously busy instead of the bursty
    QK-then-PV phases that stall each on the other.
  normalization: rec = 1/den (DVE), DRAM-bounce broadcast -> rbc;
    oT[:, j, :] = o_unnorm * rbc on GPSIMD, off the critical path.
  out = oT^T @ Wo -> [1024, 2048] fp32 rows [blk*1024:(blk+1)*1024].
"""

import os
import sys

sys.path.insert(0, "/opt/trn_rl_repo")

import numpy as np
import ml_dtypes

import concourse.bass as bass
from concourse import bacc
import concourse.mybir as mybir
import concourse.tile as tile

F32 = mybir.dt.float32
BF16 = mybir.dt.float16   # compute dtype: fp16 (same PE speed as bf16, 8x finer mantissa)
BF = np.float16

H = 2048        # hidden
T = 1024        # tokens per block
NH = 32         # q heads
NKV = 8         # kv heads
D = 64          # head dim
KT = H // 128   # 16 hidden k-tiles
TT = T // 128   # 8 token tiles
NPAIR = NH // 2  # 16 head pairs
SCALE = D ** -0.5

LAST_EXEC_TIME_NS = None
LAST_RESULTS = None
_CACHED_NC = None


def build_nc(e_bufs=5, wq_bufs=3, qp_bufs=2, qk_bufs=2, pv_bufs=2, ou_bufs=4):
    nc = bacc.Bacc("TRN2")
    xT = nc.dram_tensor("xT", [H, T], BF16, kind="ExternalInput")
    wq = nc.dram_tensor("wq", [H, H], BF16, kind="ExternalInput")
    wk = nc.dram_tensor("wk", [H, NKV * D], BF16, kind="ExternalInput")
    wv = nc.dram_tensor("wv", [H, NKV * D], BF16, kind="ExternalInput")
    wo = nc.dram_tensor("wo", [H, H], BF16, kind="ExternalInput")
    out = nc.dram_tensor("out", [T, H], F32, kind="ExternalOutput")
    scr = nc.dram_tensor("scr", [NPAIR, 2, 2, 512], F32)  # recip bounce [j, hh, n]

    xT_v = xT.ap().rearrange("(k p) t -> p k t", p=128)
    wq_v = wq.ap().rearrange("(k p) m -> p k m", p=128)
    wk_v = wk.ap().rearrange("(k p) m -> p k m", p=128)
    wv_v = wv.ap().rearrange("(k p) m -> p k m", p=128)
    wo_v = wo.ap().rearrange("(k p) m -> p k m", p=128)

    with tile.TileContext(nc) as tc:
        with (
            tc.tile_pool(name="oT", bufs=1) as oT_pool,
            tc.tile_pool(name="xTs", bufs=1) as xT_pool,
            tc.tile_pool(name="kdup", bufs=1) as kdup_pool,
            tc.tile_pool(name="vplus", bufs=1) as vplus_pool,
            tc.tile_pool(name="wos", bufs=2) as wo_pool,
            tc.tile_pool(name="wks", bufs=1) as wk_pool,  # two halves share one slot
            tc.tile_pool(name="wqs", bufs=wq_bufs) as wq_pool,
            tc.tile_pool(name="qpair", bufs=3) as qpair_pool,
            tc.tile_pool(name="e", bufs=e_bufs) as e_pool,
            tc.tile_pool(name="ou", bufs=ou_bufs) as ou_pool,
            tc.tile_pool(name="rec", bufs=2) as rec_pool,
            tc.tile_pool(name="rbc", bufs=2) as rbc_pool,
        ):
            oT = oT_pool.tile([128, KT, T], BF16)
            xTs = xT_pool.tile([128, KT, T], BF16)
            kdup = kdup_pool.tile([128, NKV, T], BF16)  # k_g^T on both halves
            vplus = vplus_pool.tile([128, TT, NKV, D + 1], BF16)
            wk_halves = [None, None]

            def dma_wk(h):
                wkh = wk_pool.tile([128, KT, 2, 128], BF16, tag="wk", name="wk")
                nc.sync.dma_start(
                    out=wkh,
                    in_=wk_v.rearrange("p k (hh mm m) -> p k hh mm m", hh=2, mm=2)[
                        :, :, h
                    ],
                )
                wk_halves[h] = wkh

            wq_tiles = [None] * (NPAIR + 1)
            qpair_tiles = [None] * (NPAIR + 1)

            def dma_wq(j):
                wq_s = wq_pool.tile([128, KT, 128], BF16, tag="wq_s", name="wq_s")
                nc.sync.dma_start(out=wq_s, in_=wq_v[:, :, 128 * j:128 * (j + 1)])
                wq_tiles[j] = wq_s
                qpair_tiles[j] = qpair_pool.tile(
                    [128, T], BF16, tag="qpair", name="qpair"
                )

            # ---- input DMA stream: per-k-tile, compute starts on tile 0 ----
            with tc.tile_pool(name="wvs", bufs=1) as wv_pool:
                wv_s = wv_pool.tile([128, KT, 512], BF16)
                for k in range(KT):
                    nc.sync.dma_start(out=wv_s[:, k:k + 1, :], in_=wv_v[:, k:k + 1, :])
                    nc.sync.dma_start(out=xTs[:, k:k + 1, :], in_=xT_v[:, k:k + 1, :])
                dma_wk(0)
                for j in range(3):
                    dma_wq(j)

                # ---- V projection, staggered k-major ----
                # Group m consumes k-tile (r - m) in round r, so group m
                # finishes at round m+15 and its psum->sbuf copy overlaps the
                # remaining groups' matmuls; round r only needs xT chunks
                # 0..r, so compute ramps with the DMA stream.
                with tc.tile_pool(name="vps", bufs=1, space="PSUM") as vps:
                    vtiles = [
                        vps.tile([128, 512], F32, tag=f"v{m}", name=f"v{m}")
                        for m in range(TT)
                    ]
                    for r in range(KT + TT - 1):
                        for m in range(TT):
                            k = r - m
                            if 0 <= k < KT:
                                nc.tensor.matmul(
                                    vtiles[m], xTs[:, k, 128 * m:128 * (m + 1)],
                                    wv_s[:, k, :],
                                    start=(k == 0), stop=(k == KT - 1),
                                )
                        m_done = r - (KT - 1)
                        if 0 <= m_done < TT:
                            nc.vector.tensor_copy(
                                vplus[:, m_done, :, 0:D],
                                vtiles[m_done].rearrange("p (h d) -> p h d", h=NKV),
                            )
                nc.vector.memset(vplus[:, :, :, D:D + 1], 1.0)

            # ---- K projection with pair-0 q projection interleaved ----
            q_ps = [None] * (NPAIR + 1)       # open q-proj psum tile per pair

            def q_mm(j, i, pool, tag):
                """Emit the i-th (of 32) q-projection matmul for pair j."""
                n, k = divmod(i, KT)
                if k == 0:
                    q_ps[j] = pool.tile([128, 512], F32, tag=tag, name="qps")
                nc.tensor.matmul(
                    q_ps[j], wq_tiles[j][:, k, :],
                    xTs[:, k, 512 * n:512 * (n + 1)],
                    start=(k == 0), stop=(k == KT - 1),
                )
                if k == KT - 1:
                    nc.vector.tensor_copy(
                        qpair_tiles[j][:, 512 * n:512 * (n + 1)], q_ps[j]
                    )

            with (
                tc.tile_pool(name="kTs", bufs=1) as kT_pool,
                tc.tile_pool(name="kps", bufs=2, space="PSUM") as kps,
            ):
                kTs = kT_pool.tile([128, 4, T], BF16)
                q0 = 0
                for m in range(4):
                    if m == 1:
                        dma_wk(1)
                    for n in range(2):
                        ps = kps.tile([128, 512], F32, tag="kps", name="kps")
                        for k in range(KT):
                            nc.tensor.matmul(
                                ps, wk_halves[m // 2][:, k, m % 2, :],
                                xTs[:, k, 512 * n:512 * (n + 1)],
                                start=(k == 0), stop=(k == KT - 1),
                            )
                        nc.vector.tensor_copy(kTs[:, m, 512 * n:512 * (n + 1)], ps)
                        for _ in range(4):
                            q_mm(0, q0, kps, "q0ps")
                            q0 += 1
                    for gg in range(2):
                        g = 2 * m + gg
                        src = kTs[64 * gg:64 * gg + 64, m, :]
                        nc.sync.dma_start(out=kdup[0:64, g, :], in_=src)
                        nc.sync.dma_start(out=kdup[64:128, g, :], in_=src)

            # ---------------- pair loop (software-pipelined) ----------------
            with (
                tc.tile_pool(name="pp", bufs=qp_bufs, space="PSUM") as pp,
                tc.tile_pool(name="qk", bufs=1, space="PSUM") as qk_pool,
                tc.tile_pool(name="pv", bufs=pv_bufs, space="PSUM") as pv_pool,
            ):
                wo_tiles = [None] * 4
                wo_s0 = wo_pool.tile([128, KT, 512], BF16, tag="wo_s")
                nc.sync.dma_start(out=wo_s0, in_=wo_v[:, :, 0:512])
                wo_tiles[0] = wo_s0

                # persistent alternating qk psum tiles: unit u writes
                # qk_ring[u % qk_bufs]; the WAR edge is exactly vs the exp
                # that read it qk_bufs units ago.
                qk_ring = [
                    qk_pool.tile([128, T], F32, tag=f"qkr{i}", name=f"qkr{i}")
                    for i in range(qk_bufs)
                ]

                e_tiles = {}                       # (j, u) -> e tile
                pv_tiles = {}                      # (j, hh) -> [pv_n0, pv_n1]
                ou_tiles = {}                      # (j, n) -> packed ou tile

                def qk_exp(j, u):
                    hh, t = divmod(u, 8)
                    g = j // 2
                    lo, hi = 64 * hh, 64 * hh + 64
                    qk = qk_ring[(16 * j + u) % qk_bufs]
                    for n in range(2):
                        nc.tensor.matmul(
                            qk[:, 512 * n:512 * (n + 1)],
                            kdup[lo:hi, g, 128 * t:128 * (t + 1)],
                            qpair_tiles[j][lo:hi, 512 * n:512 * (n + 1)],
                            start=True, stop=True,
                        )
                    e = e_pool.tile([128, T], BF16, tag="e", name="e")
                    nc.scalar.activation(
                        e, qk, mybir.ActivationFunctionType.Exp, scale=SCALE
                    )
                    e_tiles[(j, u)] = e

                def pv_mm(j, u):
                    hh, t = divmod(u, 8)
                    g = j // 2
                    if t == 0:
                        pv_tiles[(j, hh)] = [
                            pv_pool.tile([65, 512], F32, tag="pv", name=f"pv{n}")
                            for n in range(2)
                        ]
                    e = e_tiles.pop((j, u))
                    for n in range(2):
                        nc.tensor.matmul(
                            pv_tiles[(j, hh)][n], vplus[:, t, g, :],
                            e[:, 512 * n:512 * (n + 1)],
                            start=(t == 0), stop=(t == TT - 1),
                        )
                    if t == TT - 1:
                        finish_head(j, hh)

                def finish_head(j, hh):
                    """Drain pv psum for (j, hh): unnormalized o + denominator,
                    reciprocal via 128-wide DVE, DRAM-bounce broadcast, then
                    GPSIMD normalize into oT (off critical path)."""
                    lo, hi = 64 * hh, 64 * hh + 64
                    den = rec_pool.tile([1, 2 * 512], F32, tag="den", name="den")
                    for n in range(2):
                        pv = pv_tiles[(j, hh)][n]
                        if (j, n) not in ou_tiles:
                            ou_tiles[(j, n)] = ou_pool.tile(
                                [128, 512], F32, tag="ou", name=f"ou{n}"
                            )
                        nc.vector.tensor_copy(ou_tiles[(j, n)][lo:hi, :], pv[0:64, :])
                        nc.vector.tensor_copy(den[:, 512 * n:512 * (n + 1)], pv[64:65, :])
                    den_sp = rec_pool.tile([128, 8], F32, tag="den_sp", name="den_sp")
                    sp_src = bass.AP(
                        tensor=den.tensor, offset=den.offset,
                        ap=[[1, 1], [8, 128], [1, 8]],
                    )
                    nc.sync.dma_start(out=den_sp, in_=sp_src)
                    rec_sp = rec_pool.tile([128, 8], F32, tag="rec_sp", name="rec_sp")
                    nc.vector.reciprocal(rec_sp, den_sp)
                    nc.sync.dma_start(
                        out=scr.ap()[j, hh].rearrange("a b -> (a b)")
                        .rearrange("(p e) -> p e", p=128),
                        in_=rec_sp,
                    )
                    rbc = rbc_pool.tile([128, T], F32, tag="rbc", name="rbc")
                    bsrc = bass.AP(
                        tensor=scr.ap().tensor,
                        offset=(2 * j + hh) * T,
                        ap=[[0, 64], [1, 1024]],
                    )
                    nc.sync.dma_start(out=rbc[lo:hi, :], in_=bsrc)
                    for n in range(2):
                        nc.gpsimd.tensor_mul(
                            oT[lo:hi, j, 512 * n:512 * (n + 1)],
                            ou_tiles[(j, n)][lo:hi, :],
                            rbc[lo:hi, 512 * n:512 * (n + 1)],
                        )

                # steady state: 16 steps per pair; step u of pair j carries
                # QK(j,u) + exp, PV for the unit 2 steps back, and 2 matmuls
                # of pair j+1's q projection.
                for j in range(NPAIR):
                    if 3 <= j + 2 <= NPAIR - 1:
                        dma_wq(j + 2)
                    for u in range(16):
                        qk_exp(j, u)
                        s = 16 * j + u
                        if s >= 2:
                            jp, up = divmod(s - 2, 16)
                            pv_mm(jp, up)
                        if j + 1 <= NPAIR - 1:
                            q_mm(j + 1, 2 * u, pp, "pp")
                            q_mm(j + 1, 2 * u + 1, pp, "pp")
                # drain: last two pv units
                for s in range(16 * NPAIR, 16 * NPAIR + 2):
                    jp, up = divmod(s - 2, 16)
                    pv_mm(jp, up)

            # ---------------- output projection ----------------
            with (
                tc.tile_pool(name="ob", bufs=4) as ob_pool,
                tc.tile_pool(name="ops", bufs=4, space="PSUM") as ops,
            ):
                for c in range(4):
                    if wo_tiles[c] is None:
                        wo_s = wo_pool.tile([128, KT, 512], BF16, tag="wo_s")
                        nc.sync.dma_start(
                            out=wo_s, in_=wo_v[:, :, 512 * c:512 * (c + 1)]
                        )
                        wo_tiles[c] = wo_s
                    wo_s = wo_tiles[c]
                    for m in range(TT):
                        ps = ops.tile([128, 512], F32, tag="ops", name="ops")
                        for k in range(KT):
                            nc.tensor.matmul(
                                ps, oT[:, k, 128 * m:128 * (m + 1)], wo_s[:, k, :],
                                start=(k == 0), stop=(k == KT - 1),
                            )
                        ob = ob_pool.tile([128, 512], F32, tag="ob", name="ob")
                        nc.scalar.copy(ob, ps)
                        nc.sync.dma_start(
                            out=out.ap()[128 * m:128 * (m + 1), 512 * c:512 * (c + 1)],
                            in_=ob,
                        )
    nc.finalize()
    return nc


def _prep_inputs(hidden_states, Wq, Wk, Wv, Wo):
    hs = np.asarray(hidden_states, dtype=np.float32)
    B = hs.shape[0]
    # token index l = ix*2048 + sx*64 + iy*32 + sy  (bnx=2, BSX=32, bny=2, BSY=32)
    hsv = hs.reshape(B, 2, 32, 2, 32, H)  # b ix sx iy sy h
    wq_b = np.asarray(Wq, dtype=np.float32).astype(BF)
    wk_b = np.asarray(Wk, dtype=np.float32).astype(BF)
    wv_b = np.asarray(Wv, dtype=np.float32).astype(BF)
    wo_b = np.asarray(Wo, dtype=np.float32).astype(BF)
    in_maps = []
    for c in range(8):
        b, ix, iy = c // 4, (c // 2) % 2, c % 2
        x_blk = hsv[b, ix, :, iy, :, :].reshape(T, H)
        xT = np.ascontiguousarray(x_blk.T).astype(BF)
        in_maps.append({"xT": xT, "wq": wq_b, "wk": wk_b, "wv": wv_b, "wo": wo_b})
    return in_maps


def kernel(hidden_states, Wq, Wk, Wv, Wo, x_dim=64, y_dim=64):
    global LAST_EXEC_TIME_NS, LAST_RESULTS, _CACHED_NC
    assert int(x_dim) == 64 and int(y_dim) == 64

    from concourse.bass_utils import run_bass_kernel_spmd

    if _CACHED_NC is None:
        _CACHED_NC = build_nc()
    nc = _CACHED_NC

    in_maps = _prep_inputs(hidden_states, Wq, Wk, Wv, Wo)
    trace = bool(os.environ.get("BASS_TRACE"))
    res = run_bass_kernel_spmd(nc, in_maps, core_ids=list(range(8)), trace=trace)
    LAST_EXEC_TIME_NS = res.exec_time_ns
    LAST_RESULTS = res
    out = np.concatenate([r["out"] for r in res.results], axis=0)
    return np.ascontiguousarray(out.reshape(2, 4096, H).astype(np.float32))


# revision 8
# speedup vs baseline: 1.2476x; 1.0480x over previous
"""Block2D attention on 8 TRN2 NeuronCores (fp16 compute, fp32 accum).

Sharding: data-parallel over the 8 independent (b, bnx, bny) attention blocks
(B=2 x bnx=2 x bny=2), one block of T=1024 tokens per core. Blocks are fully
independent so no collectives are needed; each core runs the whole
qkv-projection -> block attention -> output projection chain for its block.

Per-core pipeline (fp16 matmuls, fp32 PSUM accumulation):
  setup: xT/wv/wk/wq streamed per k-tile up front; V projection runs k-major
         in two 4-bank PSUM passes so compute starts after the first k-tile
         lands and the K projection's PSUM frees early; K projection with
         pair-0's q projection interleaved; kT duplicated into kdup per group.
  pair loop (heads 2j, 2j+1 per pair j, kv group g=j//2), software-pipelined
  in 16 steps per pair (u = hh*8 + t):
    step u: QK(u): scoresT[t-tile, hh] = k_g @ q_h^T  (2 matmuls, psum ping-pong)
            exp(u): ACT e = exp(scale*s) -> fp16      (paced 1:1 with QK)
            PV(u-2): two accumulating matmuls into pv psum (unit lags 2 steps
                     so the exp result is ready; denominator in psum row 64)
            Q(j+1): 2 of next pair's 32 q-projection matmuls
    This keeps PE and ACT both continuously busy instead of the bursty
    QK-then-PV phases that stall each on the other.
  normalization: rec = 1/den (DVE), DRAM-bounce broadcast -> rbc;
    oT[:, j, :] = o_unnorm * rbc on GPSIMD, off the critical path.
  out = oT^T @ Wo -> [1024, 2048] fp32 rows [blk*1024:(blk+1)*1024].
"""

import os
import sys

sys.path.insert(0, "/opt/trn_rl_repo")

import numpy as np
import ml_dtypes

import concourse.bass as bass
from concourse import bacc
import concourse.mybir as mybir
import concourse.tile as tile

F32 = mybir.dt.float32
BF16 = mybir.dt.float16   # compute dtype: fp16 (same PE speed as bf16, 8x finer mantissa)
BF = np.float16

H = 2048        # hidden
T = 1024        # tokens per block
NH = 32         # q heads
NKV = 8         # kv heads
D = 64          # head dim
KT = H // 128   # 16 hidden k-tiles
TT = T // 128   # 8 token tiles
NPAIR = NH // 2  # 16 head pairs
SCALE = D ** -0.5

LAST_EXEC_TIME_NS = None
LAST_RESULTS = None
_CACHED_NC = None


def build_nc(e_bufs=5, wq_bufs=3, qp_bufs=2, qk_bufs=2, pv_bufs=2, ou_bufs=4):
    nc = bacc.Bacc("TRN2")
    xT = nc.dram_tensor("xT", [H, T], BF16, kind="ExternalInput")
    wq = nc.dram_tensor("wq", [H, H], BF16, kind="ExternalInput")
    wk = nc.dram_tensor("wk", [H, NKV * D], BF16, kind="ExternalInput")
    wv = nc.dram_tensor("wv", [H, NKV * D], BF16, kind="ExternalInput")
    wo = nc.dram_tensor("wo", [H, H], BF16, kind="ExternalInput")
    out = nc.dram_tensor("out", [T, H], F32, kind="ExternalOutput")
    scr = nc.dram_tensor("scr", [NPAIR, 2, 2, 512], F32)  # recip bounce [j, hh, n]

    xT_v = xT.ap().rearrange("(k p) t -> p k t", p=128)
    wq_v = wq.ap().rearrange("(k p) m -> p k m", p=128)
    wk_v = wk.ap().rearrange("(k p) m -> p k m", p=128)
    wv_v = wv.ap().rearrange("(k p) m -> p k m", p=128)
    wo_v = wo.ap().rearrange("(k p) m -> p k m", p=128)

    with tile.TileContext(nc) as tc:
        with (
            tc.tile_pool(name="oT", bufs=1) as oT_pool,
            tc.tile_pool(name="xTs", bufs=1) as xT_pool,
            tc.tile_pool(name="kdup", bufs=1) as kdup_pool,
            tc.tile_pool(name="vplus", bufs=1) as vplus_pool,
            tc.tile_pool(name="wos", bufs=2) as wo_pool,
            tc.tile_pool(name="wks", bufs=1) as wk_pool,  # two halves share one slot
            tc.tile_pool(name="wqs", bufs=wq_bufs) as wq_pool,
            tc.tile_pool(name="qpair", bufs=3) as qpair_pool,
            tc.tile_pool(name="e", bufs=e_bufs) as e_pool,
            tc.tile_pool(name="ou", bufs=ou_bufs) as ou_pool,
            tc.tile_pool(name="rec", bufs=2) as rec_pool,
            tc.tile_pool(name="rbc", bufs=2) as rbc_pool,
        ):
            oT = oT_pool.tile([128, KT, T], BF16)
            xTs = xT_pool.tile([128, KT, T], BF16)
            kdup = kdup_pool.tile([128, NKV, T], BF16)  # k_g^T on both halves
            vplus = vplus_pool.tile([128, TT, NKV, D + 1], BF16)
            wk_halves = [None, None]

            def dma_wk(h):
                wkh = wk_pool.tile([128, KT, 2, 128], BF16, tag="wk", name="wk")
                nc.sync.dma_start(
                    out=wkh,
                    in_=wk_v.rearrange("p k (hh mm m) -> p k hh mm m", hh=2, mm=2)[
                        :, :, h
                    ],
                )
                wk_halves[h] = wkh

            wq_tiles = [None] * (NPAIR + 1)
            qpair_tiles = [None] * (NPAIR + 1)

            def dma_wq(j):
                wq_s = wq_pool.tile([128, KT, 128], BF16, tag="wq_s", name="wq_s")
                nc.sync.dma_start(out=wq_s, in_=wq_v[:, :, 128 * j:128 * (j + 1)])
                wq_tiles[j] = wq_s
                qpair_tiles[j] = qpair_pool.tile(
                    [128, T], BF16, tag="qpair", name="qpair"
                )

            # ---- input DMA stream: per-k-tile, compute starts on tile 0 ----
            with tc.tile_pool(name="wvs", bufs=1) as wv_pool:
                wv_s = wv_pool.tile([128, KT, 512], BF16)
                # coarse chunks: each DMA trigger costs ~650ns serially on the
                # sync engine, so 12 triggers instead of 32
                for c in range(8):
                    if c % 2 == 0:
                        w = c // 2
                        nc.sync.dma_start(
                            out=wv_s[:, 4 * w:4 * (w + 1), :],
                            in_=wv_v[:, 4 * w:4 * (w + 1), :],
                        )
                    nc.sync.dma_start(
                        out=xTs[:, 2 * c:2 * (c + 1), :],
                        in_=xT_v[:, 2 * c:2 * (c + 1), :],
                    )
                dma_wk(0)
                for j in range(3):
                    dma_wq(j)

                # ---- V projection, staggered k-major ----
                # Group m consumes k-tile (r - m) in round r, so group m
                # finishes at round m+15 and its psum->sbuf copy overlaps the
                # remaining groups' matmuls; round r only needs xT chunks
                # 0..r, so compute ramps with the DMA stream.
                with tc.tile_pool(name="vps", bufs=1, space="PSUM") as vps:
                    vtiles = [
                        vps.tile([128, 512], F32, tag=f"v{m}", name=f"v{m}")
                        for m in range(TT)
                    ]
                    for r in range(KT + TT - 1):
                        for m in range(TT):
                            k = r - m
                            if 0 <= k < KT:
                                nc.tensor.matmul(
                                    vtiles[m], xTs[:, k, 128 * m:128 * (m + 1)],
                                    wv_s[:, k, :],
                                    start=(k == 0), stop=(k == KT - 1),
                                )
                        m_done = r - (KT - 1)
                        if 0 <= m_done < TT:
                            nc.vector.tensor_copy(
                                vplus[:, m_done, :, 0:D],
                                vtiles[m_done].rearrange("p (h d) -> p h d", h=NKV),
                            )
                nc.vector.memset(vplus[:, :, :, D:D + 1], 1.0)

            # ---- K projection with pair-0 q projection interleaved ----
            q_ps = [None] * (NPAIR + 1)       # open q-proj psum tile per pair

            def q_mm(j, i, pool, tag):
                """Emit the i-th (of 32) q-projection matmul for pair j."""
                n, k = divmod(i, KT)
                if k == 0:
                    q_ps[j] = pool.tile([128, 512], F32, tag=tag, name="qps")
                nc.tensor.matmul(
                    q_ps[j], wq_tiles[j][:, k, :],
                    xTs[:, k, 512 * n:512 * (n + 1)],
                    start=(k == 0), stop=(k == KT - 1),
                )
                if k == KT - 1:
                    nc.vector.tensor_copy(
                        qpair_tiles[j][:, 512 * n:512 * (n + 1)], q_ps[j]
                    )

            with (
                tc.tile_pool(name="kTs", bufs=1) as kT_pool,
                tc.tile_pool(name="kps", bufs=2, space="PSUM") as kps,
            ):
                kTs = kT_pool.tile([128, 4, T], BF16)
                q0 = 0
                for m in range(4):
                    if m == 1:
                        dma_wk(1)
                    for n in range(2):
                        ps = kps.tile([128, 512], F32, tag="kps", name="kps")
                        for k in range(KT):
                            nc.tensor.matmul(
                                ps, wk_halves[m // 2][:, k, m % 2, :],
                                xTs[:, k, 512 * n:512 * (n + 1)],
                                start=(k == 0), stop=(k == KT - 1),
                            )
                        nc.vector.tensor_copy(kTs[:, m, 512 * n:512 * (n + 1)], ps)
                        for _ in range(4):
                            q_mm(0, q0, kps, "q0ps")
                            q0 += 1
                    for gg in range(2):
                        g = 2 * m + gg
                        src = kTs[64 * gg:64 * gg + 64, m, :]
                        nc.sync.dma_start(out=kdup[0:64, g, :], in_=src)
                        nc.sync.dma_start(out=kdup[64:128, g, :], in_=src)

            # ---------------- pair loop (software-pipelined) ----------------
            with (
                tc.tile_pool(name="pp", bufs=qp_bufs, space="PSUM") as pp,
                tc.tile_pool(name="qk", bufs=1, space="PSUM") as qk_pool,
                tc.tile_pool(name="pv", bufs=pv_bufs, space="PSUM") as pv_pool,
            ):
                wo_tiles = [None] * 4
                wo_s0 = wo_pool.tile([128, KT, 512], BF16, tag="wo_s")
                nc.sync.dma_start(out=wo_s0, in_=wo_v[:, :, 0:512])
                wo_tiles[0] = wo_s0

                # persistent alternating qk psum tiles: unit u writes
                # qk_ring[u % qk_bufs]; the WAR edge is exactly vs the exp
                # that read it qk_bufs units ago.
                qk_ring = [
                    qk_pool.tile([128, T], F32, tag=f"qkr{i}", name=f"qkr{i}")
                    for i in range(qk_bufs)
                ]

                e_tiles = {}                       # (j, u) -> e tile
                pv_tiles = {}                      # (j, hh) -> [pv_n0, pv_n1]
                ou_tiles = {}                      # (j, n) -> packed ou tile

                def qk_exp(j, u):
                    hh, t = divmod(u, 8)
                    g = j // 2
                    lo, hi = 64 * hh, 64 * hh + 64
                    qk = qk_ring[(16 * j + u) % qk_bufs]
                    for n in range(2):
                        nc.tensor.matmul(
                            qk[:, 512 * n:512 * (n + 1)],
                            kdup[lo:hi, g, 128 * t:128 * (t + 1)],
                            qpair_tiles[j][lo:hi, 512 * n:512 * (n + 1)],
                            start=True, stop=True,
                        )
                    e = e_pool.tile([128, T], BF16, tag="e", name="e")
                    nc.scalar.activation(
                        e, qk, mybir.ActivationFunctionType.Exp, scale=SCALE
                    )
                    e_tiles[(j, u)] = e

                def pv_mm(j, u):
                    hh, t = divmod(u, 8)
                    g = j // 2
                    if t == 0:
                        pv_tiles[(j, hh)] = [
                            pv_pool.tile([65, 512], F32, tag="pv", name=f"pv{n}")
                            for n in range(2)
                        ]
                    e = e_tiles.pop((j, u))
                    for n in range(2):
                        nc.tensor.matmul(
                            pv_tiles[(j, hh)][n], vplus[:, t, g, :],
                            e[:, 512 * n:512 * (n + 1)],
                            start=(t == 0), stop=(t == TT - 1),
                        )
                    if t == TT - 1:
                        finish_head(j, hh)

                def finish_head(j, hh):
                    """Drain pv psum for (j, hh): unnormalized o + denominator,
                    reciprocal via 128-wide DVE, DRAM-bounce broadcast, then
                    GPSIMD normalize into oT (off critical path)."""
                    lo, hi = 64 * hh, 64 * hh + 64
                    den = rec_pool.tile([1, 2 * 512], F32, tag="den", name="den")
                    for n in range(2):
                        pv = pv_tiles[(j, hh)][n]
                        if (j, n) not in ou_tiles:
                            ou_tiles[(j, n)] = ou_pool.tile(
                                [128, 512], F32, tag="ou", name=f"ou{n}"
                            )
                        nc.vector.tensor_copy(ou_tiles[(j, n)][lo:hi, :], pv[0:64, :])
                        nc.vector.tensor_copy(den[:, 512 * n:512 * (n + 1)], pv[64:65, :])
                    den_sp = rec_pool.tile([128, 8], F32, tag="den_sp", name="den_sp")
                    sp_src = bass.AP(
                        tensor=den.tensor, offset=den.offset,
                        ap=[[1, 1], [8, 128], [1, 8]],
                    )
                    nc.sync.dma_start(out=den_sp, in_=sp_src)
                    rec_sp = rec_pool.tile([128, 8], F32, tag="rec_sp", name="rec_sp")
                    nc.vector.reciprocal(rec_sp, den_sp)
                    nc.sync.dma_start(
                        out=scr.ap()[j, hh].rearrange("a b -> (a b)")
                        .rearrange("(p e) -> p e", p=128),
                        in_=rec_sp,
                    )
                    rbc = rbc_pool.tile([128, T], F32, tag="rbc", name="rbc")
                    bsrc = bass.AP(
                        tensor=scr.ap().tensor,
                        offset=(2 * j + hh) * T,
                        ap=[[0, 64], [1, 1024]],
                    )
                    nc.sync.dma_start(out=rbc[lo:hi, :], in_=bsrc)
                    for n in range(2):
                        nc.gpsimd.tensor_mul(
                            oT[lo:hi, j, 512 * n:512 * (n + 1)],
                            ou_tiles[(j, n)][lo:hi, :],
                            rbc[lo:hi, 512 * n:512 * (n + 1)],
                        )

                # steady state: supersteps of 2 units. Emitting both QK
                # bursts adjacently hands ACT two exps per wakeup and lands
                # the qk-ring WAR release inside the PV/Q stretch, so the PE
                # runs at its matmul rate instead of locking 1:1 to exp.
                for U in range(0, 16 * NPAIR, 2):
                    j, u = divmod(U, 16)
                    if u == 0 and 3 <= j + 2 <= NPAIR - 1:
                        dma_wq(j + 2)
                    qk_exp(j, u)
                    qk_exp(j, u + 1)
                    for Up in (U - 2, U - 1):
                        if Up >= 0:
                            pv_mm(*divmod(Up, 16))
                    if j + 1 <= NPAIR - 1:
                        for i in range(4 * (u // 2), 4 * (u // 2) + 4):
                            q_mm(j + 1, i, pp, "pp")
                # drain: last two pv units
                for Up in range(16 * NPAIR - 2, 16 * NPAIR):
                    pv_mm(*divmod(Up, 16))

            # ---------------- output projection ----------------
            with (
                tc.tile_pool(name="ob", bufs=4) as ob_pool,
                tc.tile_pool(name="ops", bufs=4, space="PSUM") as ops,
            ):
                for c in range(4):
                    if wo_tiles[c] is None:
                        wo_s = wo_pool.tile([128, KT, 512], BF16, tag="wo_s")
                        nc.sync.dma_start(
                            out=wo_s, in_=wo_v[:, :, 512 * c:512 * (c + 1)]
                        )
                        wo_tiles[c] = wo_s
                    wo_s = wo_tiles[c]
                    for m in range(TT):
                        ps = ops.tile([128, 512], F32, tag="ops", name="ops")
                        for k in range(KT):
                            nc.tensor.matmul(
                                ps, oT[:, k, 128 * m:128 * (m + 1)], wo_s[:, k, :],
                                start=(k == 0), stop=(k == KT - 1),
                            )
                        ob = ob_pool.tile([128, 512], F32, tag="ob", name="ob")
                        nc.scalar.copy(ob, ps)
                        nc.sync.dma_start(
                            out=out.ap()[128 * m:128 * (m + 1), 512 * c:512 * (c + 1)],
                            in_=ob,
                        )
    nc.finalize()
    return nc


def _prep_inputs(hidden_states, Wq, Wk, Wv, Wo):
    hs = np.asarray(hidden_states, dtype=np.float32)
    B = hs.shape[0]
    # token index l = ix*2048 + sx*64 + iy*32 + sy  (bnx=2, BSX=32, bny=2, BSY=32)
    hsv = hs.reshape(B, 2, 32, 2, 32, H)  # b ix sx iy sy h
    wq_b = np.asarray(Wq, dtype=np.float32).astype(BF)
    wk_b = np.asarray(Wk, dtype=np.float32).astype(BF)
    wv_b = np.asarray(Wv, dtype=np.float32).astype(BF)
    wo_b = np.asarray(Wo, dtype=np.float32).astype(BF)
    in_maps = []
    for c in range(8):
        b, ix, iy = c // 4, (c // 2) % 2, c % 2
        x_blk = hsv[b, ix, :, iy, :, :].reshape(T, H)
        xT = np.ascontiguousarray(x_blk.T).astype(BF)
        in_maps.append({"xT": xT, "wq": wq_b, "wk": wk_b, "wv": wv_b, "wo": wo_b})
    return in_maps


def kernel(hidden_states, Wq, Wk, Wv, Wo, x_dim=64, y_dim=64):
    global LAST_EXEC_TIME_NS, LAST_RESULTS, _CACHED_NC
    assert int(x_dim) == 64 and int(y_dim) == 64

    from concourse.bass_utils import run_bass_kernel_spmd

    if _CACHED_NC is None:
        _CACHED_NC = build_nc()
    nc = _CACHED_NC

    in_maps = _prep_inputs(hidden_states, Wq, Wk, Wv, Wo)
    trace = bool(os.environ.get("BASS_TRACE"))
    res = run_bass_kernel_spmd(nc, in_maps, core_ids=list(range(8)), trace=trace)
    LAST_EXEC_TIME_NS = res.exec_time_ns
    LAST_RESULTS = res
    out = np.concatenate([r["out"] for r in res.results], axis=0)
    return np.ascontiguousarray(out.reshape(2, 4096, H).astype(np.float32))
